# revision 1
# baseline (speedup 1.0000x reference)
"""Trainium2 Bass kernel for a 2-layer spiking (Synaptic) critic network.

Math (per batch row, T=8 steps, H=128, reset-by-subtract from previous spike):
    cur1 = state @ w_fc1.T
    syn1 = a1*syn1 + cur1 + spk1 @ w_rec1.T ; mem1 = b1*mem1 + syn1 - thr1*spk1_prev
    spk1 = (mem1 > thr1) ; layer 2 analogous with inputs spk1 @ w_fc2.T + spk2 @ w_rec2.T
    out_mean = tanh(mean_t(spk2) @ w_mean.T); out_std = 1.9*sigmoid(.. @ w_std.T + 2) + .1

Kernel formulation (pure data parallel, 8 cores x 8192 rows, hidden on the
128 partitions, batch chunked CB columns):

  Work in the a^-t scaled domain so the synaptic accumulator stays resident
  in PSUM for all 8 steps with *constant* recurrent weights:
    A1_t   = sum_{tau<=t} a1^-tau (cur_tau + rec-input_tau)   (PSUM, PE-accumulated)
    M1_t   = a1^-t * mem1_t = A1_t + Wt1_t
    Wt1_t  = (b1/a1)*M1_{t-1} - St1_{t-1}                      (one fused STT op)
    St1_t  = ((M1_t > thr1*a1^-t) * thr1*a1^-(t+1))            (one chained TS op)
  The stored spike value St carries the a^-(t+1) scale, which makes the
  recurrent matmul weight w_rec1.T/thr1 step-independent; only the tiny
  feedforward weights (K=6 f1, fc2, and the [128,2] output head) need 8
  pre-scaled copies (computed on host).  Layer-2 membrane M2 is assembled by
  ScalarE (PSUM drain) + GPSIMD (add), keeping VectorE short.  Spike
  averages accumulate into a shared PSUM bank via M=2 matmuls with
  a2^(t+1)/(8*thr2) * [w_mean|w_std].T (each in-flight chunk owns partition
  pair 32*(c%3)), so tanh/sigmoid run once per chunk.

  Software pipelining: the per-step dependency chain spans four engines
  (PE A1 -> ACT z1 -> DVE M1/S1 -> PE A2 -> ACT z2 -> GPS m2 -> DVE S2 ->
  PE AO), so a single chunk runs nearly serially.  Chunks are therefore
  processed in interleaved groups of G=3: every engine emits stage X for
  all chunks of the group before stage X+1, so each cross-engine wait is
  covered by the other chunks' work.

Raw Bass (no Tile): this walrus build rejects instructions carrying more
than one attached semaphore wait ("Too many sync wait commands"), which
TileContext's scheduler emits freely.  Explicit engine blocks with
standalone wait_ge instructions sidestep the limit entirely.
"""

import os
from contextlib import ExitStack

import numpy as np

N_CORES = 8
B_TOTAL = 65536
BC = B_TOTAL // N_CORES  # 8192 rows per core
CB = 512                 # batch-column chunk (one PSUM bank)
NCHUNK = BC // CB        # 16
G = 3                    # chunks interleaved in flight
T = 8
H = 128
SD = 6

GROUPS = [list(range(g, min(g + G, NCHUNK))) for g in range(0, NCHUNK, G)]

_CACHE: dict = {}


def _schedule():
    """Precompute semaphore target values for every event, mirroring the
    emission order of each engine block exactly."""
    vA1, vA2, vAO = {}, {}, {}
    pe = 0
    for C in GROUPS:
        for t in range(T):
            for c in C:
                pe += 1
                vA1[(c, t)] = pe
            for c in C:
                pe += 1
                vA2[(c, t)] = pe
            for c in C:
                pe += 1
                vAO[(c, t)] = pe

    vW, vS1, vS2, vt2, vouts = {}, {}, {}, {}, {}
    dv = 0
    for C in GROUPS:
        for t in range(T):
            for c in C:
                dv += 1
                vW[(c, t)] = dv  # pad inc at t=0
            for c in C:
                dv += 1
                vS1[(c, t)] = dv
            for c in C:
                dv += 1
                vS2[(c, t)] = dv
        for c in C:
            dv += 1
            vt2[c] = dv
        for c in C:
            dv += 1
            vouts[c] = dv

    vz1, vz2, vsig = {}, {}, {}
    ac = 0
    for C in GROUPS:
        for t in range(T):
            for c in C:
                ac += 1
                vz1[(c, t)] = ac
            for c in C:
                ac += 1
                vz2[(c, t)] = ac
        for c in C:
            ac += 1
            vsig[c] = ac

    vm2 = {}
    gp = 1  # memset inc
    for C in GROUPS:
        for t in range(1, T):
            for c in C:
                gp += 1
                vm2[(c, t)] = gp

    N_INIT = 3 + 3 * T
    vdma_ts0, vdma_om, vdma_os = {}, {}, {}
    dm = N_INIT
    for C in GROUPS:
        for c in C:
            dm += 1
            vdma_ts0[c] = dm * 16
        for c in C:
            dm += 1
            vdma_om[c] = dm * 16
            dm += 1
            vdma_os[c] = dm * 16
    return dict(vA1=vA1, vA2=vA2, vAO=vAO, vW=vW, vS1=vS1, vS2=vS2, vt2=vt2,
                vouts=vouts, vz1=vz1, vz2=vz2, vsig=vsig, vm2=vm2,
                N_INIT=N_INIT, vdma_ts0=vdma_ts0, vdma_om=vdma_om,
                vdma_os=vdma_os)


def _build(scal):
    import concourse.bass as bass
    import concourse.mybir as mybir

    a1, b1, thr1 = scal["a1"], scal["b1"], scal["thr1"]
    a2, b2, thr2 = scal["a2"], scal["b2"], scal["thr2"]
    f32 = mybir.dt.float32
    bf16 = mybir.dt.bfloat16
    Alu = mybir.AluOpType
    Act = mybir.ActivationFunctionType

    S = _schedule()
    vA1, vA2, vAO = S["vA1"], S["vA2"], S["vAO"]
    vW, vS1, vS2 = S["vW"], S["vS1"], S["vS2"]
    vt2, vouts = S["vt2"], S["vouts"]
    vz1, vz2, vsig = S["vz1"], S["vz2"], S["vsig"]
    vm2 = S["vm2"]
    N_INIT = S["N_INIT"]
    vdma_ts0, vdma_om, vdma_os = S["vdma_ts0"], S["vdma_om"], S["vdma_os"]

    nc = bass.Bass()
    d_state = nc.declare_dram_parameter("stateT", [SD, BC], bf16, isOutput=False)
    d_w1 = nc.declare_dram_parameter("w1", [H, H], bf16, isOutput=False)
    d_r2 = nc.declare_dram_parameter("r2", [H, H], bf16, isOutput=False)
    d_f1 = nc.declare_dram_parameter("f1s", [T, SD, H], bf16, isOutput=False)
    d_w2 = nc.declare_dram_parameter("w2s", [T, H, H], bf16, isOutput=False)
    d_wo = nc.declare_dram_parameter("wos", [T, H, 2], bf16, isOutput=False)
    d_om = nc.declare_dram_parameter("out_mean", [1, BC], f32, isOutput=True)
    d_os = nc.declare_dram_parameter("out_std", [1, BC], f32, isOutput=True)

    with ExitStack() as ctx:
        E = ctx.enter_context
        sb_state = E(nc.sbuf_tensor([SD, BC], bf16))
        sb_w1 = E(nc.sbuf_tensor([H, H], bf16))
        sb_r2 = E(nc.sbuf_tensor([H, H], bf16))
        sb_f1 = E(nc.sbuf_tensor([SD, T, H], bf16))
        sb_w2 = E(nc.sbuf_tensor([H, T, H], bf16))
        sb_wo = E(nc.sbuf_tensor([H, T, 2], bf16))
        sb_two = E(nc.sbuf_tensor([1, 1], f32))

        M1 = [E(nc.sbuf_tensor(f"M1_{i}", [H, CB], bf16)) for i in range(G)]
        S1 = [E(nc.sbuf_tensor(f"S1_{i}", [H, CB], bf16)) for i in range(G)]
        W1t = [E(nc.sbuf_tensor(f"W1t_{i}", [H, CB], bf16)) for i in range(G)]
        z1 = [E(nc.sbuf_tensor(f"z1_{i}", [H, CB], bf16)) for i in range(G)]
        M2 = [E(nc.sbuf_tensor(f"M2_{i}", [H, CB], bf16)) for i in range(G)]
        S2 = [E(nc.sbuf_tensor(f"S2_{i}", [H, CB], bf16)) for i in range(G)]
        W2t = [E(nc.sbuf_tensor(f"W2t_{i}", [H, CB], bf16)) for i in range(G)]
        z2 = [E(nc.sbuf_tensor(f"z2_{i}", [H, CB], bf16)) for i in range(G)]
        t2 = [E(nc.sbuf_tensor(f"t2_{i}", [2, CB], f32)) for i in range(G)]
        ts0 = [E(nc.sbuf_tensor(f"ts0_{i}", [1, CB], f32)) for i in range(G)]
        outm = [E(nc.sbuf_tensor(f"outm_{i}", [1, CB], f32)) for i in range(G)]
        outsa = [E(nc.sbuf_tensor(f"outsa_{i}", [1, CB], f32)) for i in range(G)]
        outs2 = [E(nc.sbuf_tensor(f"outs2_{i}", [1, CB], f32)) for i in range(G)]

        A1p = [E(nc.psum_tensor(f"A1_{i}", [H, CB], f32)) for i in range(G)]
        A2p = [E(nc.psum_tensor(f"A2_{i}", [H, CB], f32)) for i in range(G)]
        AOp = E(nc.psum_tensor("AO", [H, CB], f32))  # chunk c: rows 32*(c%G)+0..1

        s_pe = E(nc.semaphore("s_pe"))
        s_dve = E(nc.semaphore("s_dve"))
        s_act = E(nc.semaphore("s_act"))
        s_gps = E(nc.semaphore("s_gps"))
        s_dma = E(nc.semaphore("s_dma"))

        block = E(nc.Block())

        @block.sync
        def _(sp):
            sp.dma_start(out=sb_state[:, :], in_=d_state[:, :]).then_inc(s_dma, 16)
            sp.dma_start(out=sb_w1[:, :], in_=d_w1[:, :]).then_inc(s_dma, 16)
            sp.dma_start(out=sb_r2[:, :], in_=d_r2[:, :]).then_inc(s_dma, 16)
            for t in range(T):
                sp.dma_start(out=sb_f1[:, t, :], in_=d_f1[t, :, :]).then_inc(s_dma, 16)
                sp.dma_start(out=sb_w2[:, t, :], in_=d_w2[t, :, :]).then_inc(s_dma, 16)
                sp.dma_start(out=sb_wo[:, t, :], in_=d_wo[t, :, :]).then_inc(s_dma, 16)
            for C in GROUPS:
                for c in C:
                    i = c % G
                    sp.wait_ge(s_dve, vt2[c])
                    sp.dma_start(out=ts0[i][:, :], in_=t2[i][1:2, :]) \
                        .then_inc(s_dma, 16)
                for c in C:
                    i = c % G
                    cs = slice(c * CB, (c + 1) * CB)
                    sp.wait_ge(s_act, vsig[c])
                    sp.dma_start(out=d_om[0:1, cs], in_=outm[i][:, :]) \
                        .then_inc(s_dma, 16)
                    sp.wait_ge(s_dve, vouts[c])
                    sp.dma_start(out=d_os[0:1, cs], in_=outs2[i][:, :]) \
                        .then_inc(s_dma, 16)

        @block.tensor
        def _(pe):
            pe.wait_ge(s_dma, N_INIT * 16)
            for C in GROUPS:
                for t in range(T):
                    last = t == T - 1
                    for c in C:
                        i = c % G
                        if t >= 1:
                            pe.wait_ge(s_dve, vS1[(c, t - 1)])
                            pe.wait_ge(s_act, vz1[(c, t - 1)])
                        elif c >= G:
                            pe.wait_ge(s_act, vz1[(c - G, T - 1)])
                        if t >= 1:
                            nc.tensor.matmul(A1p[i][:, :], sb_w1[:, :], S1[i][:, :],
                                             start=False, stop=False,
                                             skip_group_check=True)
                        nc.tensor.matmul(A1p[i][:, :], sb_f1[:, t, :],
                                         sb_state[:, c * CB:(c + 1) * CB],
                                         start=(t == 0), stop=last,
                                         skip_group_check=True) \
                            .then_inc(s_pe, 1)
                    for c in C:
                        i = c % G
                        pe.wait_ge(s_dve, vS1[(c, t)])
                        if t >= 1:
                            pe.wait_ge(s_act, vz2[(c, t - 1)])
                        elif c >= G:
                            pe.wait_ge(s_act, vz2[(c - G, T - 1)])
                        if t >= 1:
                            nc.tensor.matmul(A2p[i][:, :], sb_r2[:, :], S2[i][:, :],
                                             start=False, stop=False,
                                             skip_group_check=True)
                        nc.tensor.matmul(A2p[i][:, :], sb_w2[:, t, :], S1[i][:, :],
                                         start=(t == 0), stop=last,
                                         skip_group_check=True) \
                            .then_inc(s_pe, 1)
                    for c in C:
                        i = c % G
                        pe.wait_ge(s_dve, vS2[(c, t)])
                        nc.tensor.matmul(AOp[32 * i:32 * i + 2, :],
                                         sb_wo[:, t, :], S2[i][:, :],
                                         start=(t == 0), stop=last,
                                         skip_group_check=True) \
                            .then_inc(s_pe, 1)

        @block.vector
        def _(dve):
            for C in GROUPS:
                for t in range(T):
                    for c in C:
                        i = c % G
                        if t >= 1:
                            nc.vector.scalar_tensor_tensor(
                                out=W1t[i][:, :], in0=M1[i][:, :], scalar=b1 / a1,
                                in1=S1[i][:, :], op0=Alu.mult, op1=Alu.subtract)
                            if t >= 2:
                                dve.wait_ge(s_gps, vm2[(c, t - 1)])
                            elif c >= G:
                                dve.wait_ge(s_gps, vm2[(c - G, T - 1)])
                            nc.vector.scalar_tensor_tensor(
                                out=W2t[i][:, :], in0=M2[i][:, :], scalar=b2 / a2,
                                in1=S2[i][:, :], op0=Alu.mult, op1=Alu.subtract,
                            ).then_inc(s_dve, 1)
                        else:
                            dve.wait_ge(s_act, vz1[(c, t)])
                            nc.vector.tensor_copy(
                                out=W1t[i][0:1, 0:1], in_=M1[i][0:1, 0:1]
                            ).then_inc(s_dve, 1)
                    for c in C:
                        i = c % G
                        at1 = a1 ** (-t)
                        if t >= 1:
                            dve.wait_ge(s_act, vz1[(c, t)])
                            nc.vector.tensor_tensor(
                                out=M1[i][:, :], in0=z1[i][:, :], in1=W1t[i][:, :],
                                op=Alu.add)
                        nc.vector.tensor_scalar(
                            out=S1[i][:, :], in0=M1[i][:, :],
                            scalar1=thr1 * at1, scalar2=thr1 * at1 / a1,
                            op0=Alu.is_gt, op1=Alu.mult,
                        ).then_inc(s_dve, 1)
                    for c in C:
                        i = c % G
                        at2 = a2 ** (-t)
                        if t >= 1:
                            dve.wait_ge(s_gps, vm2[(c, t)])
                        else:
                            dve.wait_ge(s_act, vz2[(c, t)])
                        nc.vector.tensor_scalar(
                            out=S2[i][:, :], in0=M2[i][:, :],
                            scalar1=thr2 * at2, scalar2=thr2 * at2 / a2,
                            op0=Alu.is_gt, op1=Alu.mult,
                        ).then_inc(s_dve, 1)
                # group tail
                for c in C:
                    i = c % G
                    dve.wait_ge(s_pe, vAO[(C[-1], T - 1)])
                    if c >= G:
                        dve.wait_ge(s_dma, vdma_ts0[c - G])
                    nc.vector.tensor_copy(out=t2[i][:, :],
                                          in_=AOp[32 * i:32 * i + 2, :]) \
                        .then_inc(s_dve, 1)
                for c in C:
                    i = c % G
                    dve.wait_ge(s_act, vsig[c])
                    if c >= G:
                        dve.wait_ge(s_dma, vdma_os[c - G])
                    nc.vector.tensor_scalar(
                        out=outs2[i][:, :], in0=outsa[i][:, :],
                        scalar1=1.9, scalar2=0.1, op0=Alu.mult, op1=Alu.add,
                    ).then_inc(s_dve, 1)

        @block.scalar
        def _(act):
            for C in GROUPS:
                for t in range(T):
                    for c in C:
                        i = c % G
                        act.wait_ge(s_pe, vA1[(c, t)])
                        if t >= 1:
                            act.wait_ge(s_dve, vS1[(c, t - 1)])
                        elif c >= G:
                            act.wait_ge(s_dve, vt2[c - G])
                        z1out = M1[i] if t == 0 else z1[i]
                        nc.scalar.activation(out=z1out[:, :], in_=A1p[i][:, :],
                                             func=Act.Copy).then_inc(s_act, 1)
                    for c in C:
                        i = c % G
                        act.wait_ge(s_pe, vA2[(c, t)])
                        if t >= 2:
                            act.wait_ge(s_gps, vm2[(c, t - 1)])
                        elif t == 1 and c >= G:
                            act.wait_ge(s_gps, vm2[(c - G, T - 1)])
                        z2out = M2[i] if t == 0 else z2[i]
                        nc.scalar.activation(out=z2out[:, :], in_=A2p[i][:, :],
                                             func=Act.Copy).then_inc(s_act, 1)
                # group tail
                for c in C:
                    i = c % G
                    act.wait_ge(s_dve, vt2[c])
                    if c >= G:
                        act.wait_ge(s_dma, vdma_om[c - G])
                    nc.scalar.activation(out=outm[i][:, :], in_=t2[i][0:1, :],
                                         func=Act.Tanh)
                    act.wait_ge(s_dma, vdma_ts0[c])
                    if c == 0:
                        act.wait_ge(s_gps, 1)
                    nc.scalar.activation(out=outsa[i][:, :], in_=ts0[i][:, :],
                                         func=Act.Sigmoid, bias=sb_two[0:1, 0:1]) \
                        .then_inc(s_act, 1)

        @block.gpsimd
        def _(gps):
            nc.gpsimd.memset(sb_two.ap(), 2.0).then_inc(s_gps, 1)
            for C in GROUPS:
                for t in range(1, T):
                    for c in C:
                        i = c % G
                        gps.wait_ge(s_act, vz2[(c, t)])
                        gps.wait_ge(s_dve, vW[(c, t)])
                        nc.gpsimd.tensor_tensor(
                            out=M2[i][:, :], in0=z2[i][:, :], in1=W2t[i][:, :],
                            op=Alu.add).then_inc(s_gps, 1)

    return nc


def _prep(scal, w_fc1, w_rec1, w_fc2, w_rec2, w_mean, w_std):
    import ml_dtypes

    a1, b1, thr1 = scal["a1"], scal["b1"], scal["thr1"]
    a2, b2, thr2 = scal["a2"], scal["b2"], scal["thr2"]
    bf = ml_dtypes.bfloat16
    w1 = (w_rec1.T / thr1).astype(bf)
    r2 = (w_rec2.T / thr2).astype(bf)
    f1s = np.stack([(a1 ** -t) * w_fc1.T for t in range(T)]).astype(bf)
    w2s = np.stack([(a2 ** -t) * (a1 ** (t + 1)) / thr1 * w_fc2.T
                    for t in range(T)]).astype(bf)
    wo = np.concatenate([w_mean, w_std], axis=0).T / (T * thr2)  # [H, 2]
    wos = np.stack([(a2 ** (t + 1)) * wo for t in range(T)]).astype(bf)
    return w1, r2, f1s, w2s, wos


def kernel(state, w_fc1, w_rec1, w_fc2, w_rec2, w_mean, w_std,
           alpha1, beta1, thr1, alpha2, beta2, thr2):
    import ml_dtypes
    from concourse.bass_utils import run_bass_kernel_spmd

    state = np.asarray(state, dtype=np.float32)
    scal = {
        "a1": float(np.clip(np.asarray(alpha1, dtype=np.float64), 1e-6, 1.0)),
        "b1": float(np.clip(np.asarray(beta1, dtype=np.float64), 0.0, 1.0)),
        "thr1": float(np.asarray(thr1, dtype=np.float64)),
        "a2": float(np.clip(np.asarray(alpha2, dtype=np.float64), 1e-6, 1.0)),
        "b2": float(np.clip(np.asarray(beta2, dtype=np.float64), 0.0, 1.0)),
        "thr2": float(np.asarray(thr2, dtype=np.float64)),
    }

    key = tuple(sorted(scal.items()))
    if key not in _CACHE:
        _CACHE[key] = _build(scal)
    nc = _CACHE[key]

    w1, r2, f1s, w2s, wos = _prep(
        scal,
        np.asarray(w_fc1, np.float32), np.asarray(w_rec1, np.float32),
        np.asarray(w_fc2, np.float32), np.asarray(w_rec2, np.float32),
        np.asarray(w_mean, np.float32), np.asarray(w_std, np.float32),
    )
    stateT = state.T.astype(ml_dtypes.bfloat16)  # [6, B_TOTAL]

    in_maps = []
    for c in range(N_CORES):
        in_maps.append({
            "stateT": np.ascontiguousarray(stateT[:, c * BC : (c + 1) * BC]),
            "w1": w1, "r2": r2, "f1s": f1s, "w2s": w2s, "wos": wos,
        })

    res = run_bass_kernel_spmd(nc, in_maps, core_ids=list(range(N_CORES)),
                               trace=bool(int(os.environ.get("SNN_TRACE", "0"))))
    kernel.last_results = res
    vm = np.concatenate([res.results[c]["out_mean"] for c in range(N_CORES)], axis=1)
    vs = np.concatenate([res.results[c]["out_std"] for c in range(N_CORES)], axis=1)
    return vm.reshape(-1, 1), vs.reshape(-1, 1)



# revision 9
# speedup vs baseline: 1.7281x; 1.7281x over previous
"""Trainium2 Bass kernel for a 2-layer spiking (Synaptic) critic network.

Math (per batch row, T=8 steps, H=128, reset-by-subtract from previous spike):
    cur1 = state @ w_fc1.T
    syn1 = a1*syn1 + cur1 + spk1 @ w_rec1.T ; mem1 = b1*mem1 + syn1 - thr1*spk1_prev
    spk1 = (mem1 > thr1) ; layer 2 analogous with inputs spk1 @ w_fc2.T + spk2 @ w_rec2.T
    out_mean = tanh(mean_t(spk2) @ w_mean.T); out_std = 1.9*sigmoid(.. @ w_std.T + 2) + .1

Fast path (requires beta == alpha, which clip-holds for the shipped scalars):

  Work in the a^-t scaled domain M_t = a^-t * mem_t.  With b == a the decay
  multiplier (b/a) is exactly 1, so the scaled membrane is a PURE RUNNING SUM
  and can live in PSUM for all 8 steps, accumulated by the PE:
      M1_t = M1_{t-1} + a1^-t*cur1 + spk1_{t-1} @ [a1^-t (w_rec1.T - thr1 I)]
  The reset-by-subtract is folded into the recurrent weight matrix (the -thr*I
  term), so no elementwise membrane updates exist at all.  Per step each layer
  needs ONLY its matmuls plus one spike-extraction op reading PSUM directly:

    spk1 via ScalarE:  Sh1 = Sign(M1psum + C1adj_t)  in {-1,+1}
      (sign-encoded spikes: spk = (Sh1+1)/2; consumers use half-weights W/2 and
       the constant 1/2-column-sum corrections are folded -- cumulatively, in
       f64 on the host -- into the comparison constants C1adj/C2adj, which are
       per-partition f32 vectors.  Nothing else ever reads M, so the correction
       never needs to be materialized in PSUM.)
    spk2 via VectorE:  S2 = (M2psum + C2adj_t) > 0    in {0,1}

  The output head accumulates  AO = sum_t S2_t @ ([w_mean|w_std].T / 8)  into a
  shared PSUM bank (constant weights; chunk i owns partition pair 32i..32i+1),
  so tanh/sigmoid run once per chunk at the group tail.

  Engine load per chunk-step: PE 5 matmuls (f1, W1h, W2h, R2, wo; all N=512),
  ACT 1 Sign, DVE 1 tensor_scalar.  The PE is the top engine by ~2x, which
  keeps it densely busy and HAM-warm (2.4 GHz) instead of oscillating at the
  cold 1.2 GHz clock like the previous formulation.

  Chunks are processed in interleaved groups of G=3 (each chunk owns M1/M2
  PSUM banks; 3*2+1 = 7 of 8 banks) so every cross-engine wait is covered by
  the other chunks' work.

Raw Bass (no Tile): this walrus build rejects instructions carrying more than
one attached semaphore wait; explicit engine blocks with standalone wait_ge
instructions sidestep the limit.
"""

import os
from contextlib import ExitStack

import numpy as np

N_CORES = 8
B_TOTAL = 65536
BC = B_TOTAL // N_CORES  # 8192 rows per core
CB = 512                 # batch-column chunk (one PSUM bank)
NCHUNK = BC // CB        # 16
G = 3                    # chunks interleaved in flight
T = 8
H = 128
SD = 6

GROUPS = [list(range(g, min(g + G, NCHUNK))) for g in range(0, NCHUNK, G)]

# FBIG column layout: [f1 stack T*H | state BC]
FB_W = T * H + BC
# WBIG column layout: [W1h T*H | W2h T*H | R2 T*H | wo 2]
WB_W = 3 * T * H + 2
# CONSTS column layout: [C2adj T | C1adj T | 2.0]
CN_W = 2 * T + 1

_CACHE: dict = {}


# ───────────────────────────── fast path ─────────────────────────────


def _schedule_fast():
    """Semaphore target values for every event, mirroring emission order."""
    vA1, vA2, vAO = {}, {}, {}
    pe = 0
    for C in GROUPS:
        for t in range(T):
            for c in C:
                pe += 1
                vA1[(c, t)] = pe
            for c in C:
                pe += 1
                vA2[(c, t)] = pe
            for c in C:
                pe += 1
                vAO[(c, t)] = pe

    vS1h, vtail = {}, {}
    ac = 0
    for C in GROUPS:
        for t in range(T):
            for c in C:
                ac += 1
                vS1h[(c, t)] = ac
        for c in C:
            ac += 1
            vtail[c] = ac

    vS2, vt2, vouts = {}, {}, {}
    dv = 0
    for C in GROUPS:
        for t in range(T):
            for c in C:
                dv += 1
                vS2[(c, t)] = dv
        for c in C:
            dv += 1
            vt2[c] = dv
        for c in C:
            dv += 1
            vouts[c] = dv

    N_INIT = 3  # fbig, wbig, consts
    vdma_ts0, vdma_om, vdma_os = {}, {}, {}
    dm = N_INIT
    for C in GROUPS:
        for c in C:
            dm += 1
            vdma_ts0[c] = dm * 16
        for c in C:
            dm += 1
            vdma_om[c] = dm * 16
            dm += 1
            vdma_os[c] = dm * 16
    return dict(vA1=vA1, vA2=vA2, vAO=vAO, vS1h=vS1h, vtail=vtail, vS2=vS2,
                vt2=vt2, vouts=vouts, vdma_ts0=vdma_ts0, vdma_om=vdma_om,
                vdma_os=vdma_os)


def _build_fast():
    import concourse.bass as bass
    import concourse.mybir as mybir

    f32 = mybir.dt.float32
    bf16 = mybir.dt.bfloat16
    Alu = mybir.AluOpType
    Act = mybir.ActivationFunctionType

    S = _schedule_fast()
    vA1, vA2, vAO = S["vA1"], S["vA2"], S["vAO"]
    vS1h, vtail = S["vS1h"], S["vtail"]
    vS2, vt2, vouts = S["vS2"], S["vt2"], S["vouts"]
    vdma_ts0, vdma_om, vdma_os = S["vdma_ts0"], S["vdma_om"], S["vdma_os"]

    nc = bass.Bass()
    d_fbig = nc.declare_dram_parameter("fbig", [SD, FB_W], bf16, isOutput=False)
    d_wbig = nc.declare_dram_parameter("wbig", [H, WB_W], bf16, isOutput=False)
    d_cn = nc.declare_dram_parameter("consts", [H, CN_W], f32, isOutput=False)
    d_om = nc.declare_dram_parameter("out_mean", [1, BC], f32, isOutput=True)
    d_os = nc.declare_dram_parameter("out_std", [1, BC], f32, isOutput=True)

    with ExitStack() as ctx:
        E = ctx.enter_context
        fb = E(nc.sbuf_tensor([SD, FB_W], bf16))
        wb = E(nc.sbuf_tensor([H, WB_W], bf16))
        cn = E(nc.sbuf_tensor([H, CN_W], f32))

        Sh1 = [E(nc.sbuf_tensor(f"Sh1_{i}", [H, CB], bf16)) for i in range(G)]
        S2 = [E(nc.sbuf_tensor(f"S2_{i}", [H, CB], bf16)) for i in range(G)]
        t2 = [E(nc.sbuf_tensor(f"t2_{i}", [2, CB], f32)) for i in range(G)]
        ts0 = [E(nc.sbuf_tensor(f"ts0_{i}", [1, CB], f32)) for i in range(G)]
        outm = [E(nc.sbuf_tensor(f"outm_{i}", [1, CB], f32)) for i in range(G)]
        sa = [E(nc.sbuf_tensor(f"sa_{i}", [1, CB], f32)) for i in range(G)]
        outs = [E(nc.sbuf_tensor(f"outs_{i}", [1, CB], f32)) for i in range(G)]

        M1p = [E(nc.psum_tensor(f"M1_{i}", [H, CB], f32)) for i in range(G)]
        M2p = [E(nc.psum_tensor(f"M2_{i}", [H, CB], f32)) for i in range(G)]
        AOp = E(nc.psum_tensor("AO", [H, CB], f32))  # chunk i: rows 32i..32i+1

        s_pe = E(nc.semaphore("s_pe"))
        s_dve = E(nc.semaphore("s_dve"))
        s_act = E(nc.semaphore("s_act"))
        s_dma = E(nc.semaphore("s_dma"))

        def w1h(t):
            return wb[:, t * H:(t + 1) * H]

        def w2h(t):
            return wb[:, T * H + t * H:T * H + (t + 1) * H]

        def r2w(t):
            return wb[:, 2 * T * H + t * H:2 * T * H + (t + 1) * H]

        wo = wb[:, 3 * T * H:3 * T * H + 2]

        def f1w(t):
            return fb[:, t * H:(t + 1) * H]

        def st(c):
            return fb[:, T * H + c * CB:T * H + (c + 1) * CB]

        block = E(nc.Block())

        @block.sync
        def _(sp):
            sp.dma_start(out=fb[:, :], in_=d_fbig[:, :]).then_inc(s_dma, 16)
            sp.dma_start(out=wb[:, :], in_=d_wbig[:, :]).then_inc(s_dma, 16)
            sp.dma_start(out=cn[:, :], in_=d_cn[:, :]).then_inc(s_dma, 16)
            for C in GROUPS:
                for c in C:
                    i = c % G
                    sp.wait_ge(s_dve, vt2[c])
                    sp.dma_start(out=ts0[i][:, :], in_=t2[i][1:2, :]) \
                        .then_inc(s_dma, 16)
                for c in C:
                    i = c % G
                    cs = slice(c * CB, (c + 1) * CB)
                    sp.wait_ge(s_act, vtail[c])
                    sp.dma_start(out=d_om[0:1, cs], in_=outm[i][:, :]) \
                        .then_inc(s_dma, 16)
                    sp.wait_ge(s_dve, vouts[c])
                    sp.dma_start(out=d_os[0:1, cs], in_=outs[i][:, :]) \
                        .then_inc(s_dma, 16)

        @block.tensor
        def _(pe):
            for C in GROUPS:
                for t in range(T):
                    last = t == T - 1
                    for c in C:
                        i = c % G
                        if t >= 1:
                            pe.wait_ge(s_act, vS1h[(c, t - 1)])
                        elif c >= G:
                            pe.wait_ge(s_act, vS1h[(c - G, T - 1)])
                        elif c == 0:
                            pe.wait_ge(s_dma, 16)
                        if t >= 1:
                            nc.tensor.matmul(M1p[i][:, :], w1h(t), Sh1[i][:, :],
                                             start=False, stop=False,
                                             skip_group_check=True)
                        nc.tensor.matmul(M1p[i][:, :], f1w(t), st(c),
                                         start=(t == 0), stop=last,
                                         skip_group_check=True) \
                            .then_inc(s_pe, 1)
                    for c in C:
                        i = c % G
                        pe.wait_ge(s_act, vS1h[(c, t)])
                        if t >= 1:
                            pe.wait_ge(s_dve, vS2[(c, t - 1)])
                        elif c >= G:
                            pe.wait_ge(s_dve, vS2[(c - G, T - 1)])
                        elif c == 0:
                            pe.wait_ge(s_dma, 32)
                        if t >= 1:
                            nc.tensor.matmul(M2p[i][:, :], w2h(t), Sh1[i][:, :],
                                             start=False, stop=False,
                                             skip_group_check=True)
                            nc.tensor.matmul(M2p[i][:, :], r2w(t), S2[i][:, :],
                                             start=False, stop=last,
                                             skip_group_check=True) \
                                .then_inc(s_pe, 1)
                        else:
                            nc.tensor.matmul(M2p[i][:, :], w2h(t), Sh1[i][:, :],
                                             start=True, stop=False,
                                             skip_group_check=True) \
                                .then_inc(s_pe, 1)
                    for c in C:
                        i = c % G
                        pe.wait_ge(s_dve, vS2[(c, t)])
                        nc.tensor.matmul(AOp[32 * i:32 * i + 2, :],
                                         wo, S2[i][:, :],
                                         start=(t == 0), stop=last,
                                         skip_group_check=True) \
                            .then_inc(s_pe, 1)

        @block.scalar
        def _(act):
            act.wait_ge(s_dma, 48)
            for C in GROUPS:
                for t in range(T):
                    for c in C:
                        i = c % G
                        act.wait_ge(s_pe, vA1[(c, t)])
                        nc.scalar.activation(
                            out=Sh1[i][:, :], in_=M1p[i][:, :], func=Act.Sign,
                            bias=cn[:, T + t:T + t + 1]).then_inc(s_act, 1)
                for c in C:
                    i = c % G
                    act.wait_ge(s_dve, vt2[c])
                    if c >= G:
                        act.wait_ge(s_dma, vdma_om[c - G])
                    nc.scalar.activation(out=outm[i][:, :], in_=t2[i][0:1, :],
                                         func=Act.Tanh)
                    act.wait_ge(s_dma, vdma_ts0[c])
                    if c >= G:
                        act.wait_ge(s_dve, vouts[c - G])
                    nc.scalar.activation(out=sa[i][:, :], in_=ts0[i][:, :],
                                         func=Act.Sigmoid,
                                         bias=cn[0:1, 2 * T:2 * T + 1]) \
                        .then_inc(s_act, 1)

        @block.vector
        def _(dve):
            dve.wait_ge(s_dma, 48)
            for C in GROUPS:
                for t in range(T):
                    for c in C:
                        i = c % G
                        dve.wait_ge(s_pe, vA2[(c, t)])
                        nc.vector.tensor_scalar(
                            out=S2[i][:, :], in0=M2p[i][:, :],
                            scalar1=cn[:, t:t + 1], scalar2=0.0,
                            op0=Alu.add, op1=Alu.is_gt).then_inc(s_dve, 1)
                for c in C:
                    i = c % G
                    dve.wait_ge(s_pe, vAO[(c, T - 1)])
                    if c >= G:
                        dve.wait_ge(s_dma, vdma_ts0[c - G])
                        dve.wait_ge(s_act, vtail[c - G])
                    nc.vector.tensor_copy(out=t2[i][:, :],
                                          in_=AOp[32 * i:32 * i + 2, :]) \
                        .then_inc(s_dve, 1)
                for c in C:
                    i = c % G
                    dve.wait_ge(s_act, vtail[c])
                    if c >= G:
                        dve.wait_ge(s_dma, vdma_os[c - G])
                    nc.vector.tensor_scalar(
                        out=outs[i][:, :], in0=sa[i][:, :],
                        scalar1=1.9, scalar2=0.1, op0=Alu.mult, op1=Alu.add,
                    ).then_inc(s_dve, 1)

    return nc


def _prep_fast(scal, w_fc1, w_rec1, w_fc2, w_rec2, w_mean, w_std):
    """Host-side weight/constant packing (f64 precision)."""
    import ml_dtypes

    bf = ml_dtypes.bfloat16
    a1, thr1 = scal["a1"], scal["thr1"]
    a2, thr2 = scal["a2"], scal["thr2"]
    Iden = np.eye(H, dtype=np.float64)

    w1T = w_rec1.T.astype(np.float64)
    w2T = w_fc2.T.astype(np.float64)
    r2T = w_rec2.T.astype(np.float64)
    f1T = w_fc1.T.astype(np.float64)

    wbig = np.zeros((H, WB_W), dtype=np.float64)
    consts = np.zeros((H, CN_W), dtype=np.float64)
    fweights = np.zeros((SD, T * H), dtype=np.float64)

    C1 = np.zeros(H, dtype=np.float64)
    C2 = np.zeros(H, dtype=np.float64)
    for t in range(T):
        s1 = a1 ** (-t)
        s2 = a2 ** (-t)
        fweights[:, t * H:(t + 1) * H] = s1 * f1T
        if t >= 1:
            W1 = s1 * (w1T - thr1 * Iden)
            wbig[:, t * H:(t + 1) * H] = 0.5 * W1
            C1 = C1 + 0.5 * W1.sum(axis=0)
            wbig[:, 2 * T * H + t * H:2 * T * H + (t + 1) * H] = \
                s2 * (r2T - thr2 * Iden)
        W2 = s2 * w2T
        wbig[:, T * H + t * H:T * H + (t + 1) * H] = 0.5 * W2
        C2 = C2 + 0.5 * W2.sum(axis=0)
        consts[:, t] = C2 - thr2 * s2
        consts[:, T + t] = C1 - thr1 * s1
    wbig[:, 3 * T * H:3 * T * H + 2] = \
        np.concatenate([w_mean, w_std], axis=0).T / T
    consts[:, 2 * T] = 2.0

    return (wbig.astype(bf), fweights.astype(bf), consts.astype(np.float32))


# ──────────────────────────── legacy path ────────────────────────────
# General (beta != alpha) fallback: the previous pipelined formulation.


def _schedule_legacy():
    vA1, vA2, vAO = {}, {}, {}
    pe = 0
    for C in GROUPS:
        for t in range(T):
            for c in C:
                pe += 1
                vA1[(c, t)] = pe
            for c in C:
                pe += 1
                vA2[(c, t)] = pe
            for c in C:
                pe += 1
                vAO[(c, t)] = pe

    vW, vS1, vS2, vt2, vouts = {}, {}, {}, {}, {}
    dv = 0
    for C in GROUPS:
        for t in range(T):
            for c in C:
                dv += 1
                vW[(c, t)] = dv
            for c in C:
                dv += 1
                vS1[(c, t)] = dv
            for c in C:
                dv += 1
                vS2[(c, t)] = dv
        for c in C:
            dv += 1
            vt2[c] = dv
        for c in C:
            dv += 1
            vouts[c] = dv

    vz1, vz2, vsig = {}, {}, {}
    ac = 0
    for C in GROUPS:
        for t in range(T):
            for c in C:
                ac += 1
                vz1[(c, t)] = ac
            for c in C:
                ac += 1
                vz2[(c, t)] = ac
        for c in C:
            ac += 1
            vsig[c] = ac

    vm2 = {}
    gp = 1
    for C in GROUPS:
        for t in range(1, T):
            for c in C:
                gp += 1
                vm2[(c, t)] = gp

    N_INIT = 3 + 3 * T
    vdma_ts0, vdma_om, vdma_os = {}, {}, {}
    dm = N_INIT
    for C in GROUPS:
        for c in C:
            dm += 1
            vdma_ts0[c] = dm * 16
        for c in C:
            dm += 1
            vdma_om[c] = dm * 16
            dm += 1
            vdma_os[c] = dm * 16
    return dict(vA1=vA1, vA2=vA2, vAO=vAO, vW=vW, vS1=vS1, vS2=vS2, vt2=vt2,
                vouts=vouts, vz1=vz1, vz2=vz2, vsig=vsig, vm2=vm2,
                N_INIT=N_INIT, vdma_ts0=vdma_ts0, vdma_om=vdma_om,
                vdma_os=vdma_os)


def _build_legacy(scal):
    import concourse.bass as bass
    import concourse.mybir as mybir

    a1, b1, thr1 = scal["a1"], scal["b1"], scal["thr1"]
    a2, b2, thr2 = scal["a2"], scal["b2"], scal["thr2"]
    f32 = mybir.dt.float32
    bf16 = mybir.dt.bfloat16
    Alu = mybir.AluOpType
    Act = mybir.ActivationFunctionType

    S = _schedule_legacy()
    vA1, vA2, vAO = S["vA1"], S["vA2"], S["vAO"]
    vW, vS1, vS2 = S["vW"], S["vS1"], S["vS2"]
    vt2, vouts = S["vt2"], S["vouts"]
    vz1, vz2, vsig = S["vz1"], S["vz2"], S["vsig"]
    vm2 = S["vm2"]
    N_INIT = S["N_INIT"]
    vdma_ts0, vdma_om, vdma_os = S["vdma_ts0"], S["vdma_om"], S["vdma_os"]

    nc = bass.Bass()
    d_state = nc.declare_dram_parameter("stateT", [SD, BC], bf16, isOutput=False)
    d_w1 = nc.declare_dram_parameter("w1", [H, H], bf16, isOutput=False)
    d_r2 = nc.declare_dram_parameter("r2", [H, H], bf16, isOutput=False)
    d_f1 = nc.declare_dram_parameter("f1s", [T, SD, H], bf16, isOutput=False)
    d_w2 = nc.declare_dram_parameter("w2s", [T, H, H], bf16, isOutput=False)
    d_wo = nc.declare_dram_parameter("wos", [T, H, 2], bf16, isOutput=False)
    d_om = nc.declare_dram_parameter("out_mean", [1, BC], f32, isOutput=True)
    d_os = nc.declare_dram_parameter("out_std", [1, BC], f32, isOutput=True)

    with ExitStack() as ctx:
        E = ctx.enter_context
        sb_state = E(nc.sbuf_tensor([SD, BC], bf16))
        sb_w1 = E(nc.sbuf_tensor([H, H], bf16))
        sb_r2 = E(nc.sbuf_tensor([H, H], bf16))
        sb_f1 = E(nc.sbuf_tensor([SD, T, H], bf16))
        sb_w2 = E(nc.sbuf_tensor([H, T, H], bf16))
        sb_wo = E(nc.sbuf_tensor([H, T, 2], bf16))
        sb_two = E(nc.sbuf_tensor([1, 1], f32))

        M1 = [E(nc.sbuf_tensor(f"M1_{i}", [H, CB], bf16)) for i in range(G)]
        S1 = [E(nc.sbuf_tensor(f"S1_{i}", [H, CB], bf16)) for i in range(G)]
        W1t = [E(nc.sbuf_tensor(f"W1t_{i}", [H, CB], bf16)) for i in range(G)]
        z1 = [E(nc.sbuf_tensor(f"z1_{i}", [H, CB], bf16)) for i in range(G)]
        M2 = [E(nc.sbuf_tensor(f"M2_{i}", [H, CB], bf16)) for i in range(G)]
        S2 = [E(nc.sbuf_tensor(f"S2_{i}", [H, CB], bf16)) for i in range(G)]
        W2t = [E(nc.sbuf_tensor(f"W2t_{i}", [H, CB], bf16)) for i in range(G)]
        z2 = [E(nc.sbuf_tensor(f"z2_{i}", [H, CB], bf16)) for i in range(G)]
        t2 = [E(nc.sbuf_tensor(f"t2_{i}", [2, CB], f32)) for i in range(G)]
        ts0 = [E(nc.sbuf_tensor(f"ts0_{i}", [1, CB], f32)) for i in range(G)]
        outm = [E(nc.sbuf_tensor(f"outm_{i}", [1, CB], f32)) for i in range(G)]
        outsa = [E(nc.sbuf_tensor(f"outsa_{i}", [1, CB], f32)) for i in range(G)]
        outs2 = [E(nc.sbuf_tensor(f"outs2_{i}", [1, CB], f32)) for i in range(G)]

        A1p = [E(nc.psum_tensor(f"A1_{i}", [H, CB], f32)) for i in range(G)]
        A2p = [E(nc.psum_tensor(f"A2_{i}", [H, CB], f32)) for i in range(G)]
        AOp = E(nc.psum_tensor("AO", [H, CB], f32))

        s_pe = E(nc.semaphore("s_pe"))
        s_dve = E(nc.semaphore("s_dve"))
        s_act = E(nc.semaphore("s_act"))
        s_gps = E(nc.semaphore("s_gps"))
        s_dma = E(nc.semaphore("s_dma"))

        block = E(nc.Block())

        @block.sync
        def _(sp):
            sp.dma_start(out=sb_state[:, :], in_=d_state[:, :]).then_inc(s_dma, 16)
            sp.dma_start(out=sb_w1[:, :], in_=d_w1[:, :]).then_inc(s_dma, 16)
            sp.dma_start(out=sb_r2[:, :], in_=d_r2[:, :]).then_inc(s_dma, 16)
            for t in range(T):
                sp.dma_start(out=sb_f1[:, t, :], in_=d_f1[t, :, :]).then_inc(s_dma, 16)
                sp.dma_start(out=sb_w2[:, t, :], in_=d_w2[t, :, :]).then_inc(s_dma, 16)
                sp.dma_start(out=sb_wo[:, t, :], in_=d_wo[t, :, :]).then_inc(s_dma, 16)
            for C in GROUPS:
                for c in C:
                    i = c % G
                    sp.wait_ge(s_dve, vt2[c])
                    sp.dma_start(out=ts0[i][:, :], in_=t2[i][1:2, :]) \
                        .then_inc(s_dma, 16)
                for c in C:
                    i = c % G
                    cs = slice(c * CB, (c + 1) * CB)
                    sp.wait_ge(s_act, vsig[c])
                    sp.dma_start(out=d_om[0:1, cs], in_=outm[i][:, :]) \
                        .then_inc(s_dma, 16)
                    sp.wait_ge(s_dve, vouts[c])
                    sp.dma_start(out=d_os[0:1, cs], in_=outs2[i][:, :]) \
                        .then_inc(s_dma, 16)

        @block.tensor
        def _(pe):
            pe.wait_ge(s_dma, N_INIT * 16)
            for C in GROUPS:
                for t in range(T):
                    last = t == T - 1
                    for c in C:
                        i = c % G
                        if t >= 1:
                            pe.wait_ge(s_dve, vS1[(c, t - 1)])
                            pe.wait_ge(s_act, vz1[(c, t - 1)])
                        elif c >= G:
                            pe.wait_ge(s_act, vz1[(c - G, T - 1)])
                        if t >= 1:
                            nc.tensor.matmul(A1p[i][:, :], sb_w1[:, :], S1[i][:, :],
                                             start=False, stop=False,
                                             skip_group_check=True)
                        nc.tensor.matmul(A1p[i][:, :], sb_f1[:, t, :],
                                         sb_state[:, c * CB:(c + 1) * CB],
                                         start=(t == 0), stop=last,
                                         skip_group_check=True) \
                            .then_inc(s_pe, 1)
                    for c in C:
                        i = c % G
                        pe.wait_ge(s_dve, vS1[(c, t)])
                        if t >= 1:
                            pe.wait_ge(s_act, vz2[(c, t - 1)])
                        elif c >= G:
                            pe.wait_ge(s_act, vz2[(c - G, T - 1)])
                        if t >= 1:
                            nc.tensor.matmul(A2p[i][:, :], sb_r2[:, :], S2[i][:, :],
                                             start=False, stop=False,
                                             skip_group_check=True)
                        nc.tensor.matmul(A2p[i][:, :], sb_w2[:, t, :], S1[i][:, :],
                                         start=(t == 0), stop=last,
                                         skip_group_check=True) \
                            .then_inc(s_pe, 1)
                    for c in C:
                        i = c % G
                        pe.wait_ge(s_dve, vS2[(c, t)])
                        nc.tensor.matmul(AOp[32 * i:32 * i + 2, :],
                                         sb_wo[:, t, :], S2[i][:, :],
                                         start=(t == 0), stop=last,
                                         skip_group_check=True) \
                            .then_inc(s_pe, 1)

        @block.vector
        def _(dve):
            for C in GROUPS:
                for t in range(T):
                    for c in C:
                        i = c % G
                        if t >= 1:
                            nc.vector.scalar_tensor_tensor(
                                out=W1t[i][:, :], in0=M1[i][:, :], scalar=b1 / a1,
                                in1=S1[i][:, :], op0=Alu.mult, op1=Alu.subtract)
                            if t >= 2:
                                dve.wait_ge(s_gps, vm2[(c, t - 1)])
                            elif c >= G:
                                dve.wait_ge(s_gps, vm2[(c - G, T - 1)])
                            nc.vector.scalar_tensor_tensor(
                                out=W2t[i][:, :], in0=M2[i][:, :], scalar=b2 / a2,
                                in1=S2[i][:, :], op0=Alu.mult, op1=Alu.subtract,
                            ).then_inc(s_dve, 1)
                        else:
                            dve.wait_ge(s_act, vz1[(c, t)])
                            nc.vector.tensor_copy(
                                out=W1t[i][0:1, 0:1], in_=M1[i][0:1, 0:1]
                            ).then_inc(s_dve, 1)
                    for c in C:
                        i = c % G
                        at1 = a1 ** (-t)
                        if t >= 1:
                            dve.wait_ge(s_act, vz1[(c, t)])
                            nc.vector.tensor_tensor(
                                out=M1[i][:, :], in0=z1[i][:, :], in1=W1t[i][:, :],
                                op=Alu.add)
                        nc.vector.tensor_scalar(
                            out=S1[i][:, :], in0=M1[i][:, :],
                            scalar1=thr1 * at1, scalar2=thr1 * at1 / a1,
                            op0=Alu.is_gt, op1=Alu.mult,
                        ).then_inc(s_dve, 1)
                    for c in C:
                        i = c % G
                        at2 = a2 ** (-t)
                        if t >= 1:
                            dve.wait_ge(s_gps, vm2[(c, t)])
                        else:
                            dve.wait_ge(s_act, vz2[(c, t)])
                        nc.vector.tensor_scalar(
                            out=S2[i][:, :], in0=M2[i][:, :],
                            scalar1=thr2 * at2, scalar2=thr2 * at2 / a2,
                            op0=Alu.is_gt, op1=Alu.mult,
                        ).then_inc(s_dve, 1)
                for c in C:
                    i = c % G
                    dve.wait_ge(s_pe, vAO[(C[-1], T - 1)])
                    if c >= G:
                        dve.wait_ge(s_dma, vdma_ts0[c - G])
                    nc.vector.tensor_copy(out=t2[i][:, :],
                                          in_=AOp[32 * i:32 * i + 2, :]) \
                        .then_inc(s_dve, 1)
                for c in C:
                    i = c % G
                    dve.wait_ge(s_act, vsig[c])
                    if c >= G:
                        dve.wait_ge(s_dma, vdma_os[c - G])
                    nc.vector.tensor_scalar(
                        out=outs2[i][:, :], in0=outsa[i][:, :],
                        scalar1=1.9, scalar2=0.1, op0=Alu.mult, op1=Alu.add,
                    ).then_inc(s_dve, 1)

        @block.scalar
        def _(act):
            for C in GROUPS:
                for t in range(T):
                    for c in C:
                        i = c % G
                        act.wait_ge(s_pe, vA1[(c, t)])
                        if t >= 1:
                            act.wait_ge(s_dve, vS1[(c, t - 1)])
                        elif c >= G:
                            act.wait_ge(s_dve, vt2[c - G])
                        z1out = M1[i] if t == 0 else z1[i]
                        nc.scalar.activation(out=z1out[:, :], in_=A1p[i][:, :],
                                             func=Act.Copy).then_inc(s_act, 1)
                    for c in C:
                        i = c % G
                        act.wait_ge(s_pe, vA2[(c, t)])
                        if t >= 2:
                            act.wait_ge(s_gps, vm2[(c, t - 1)])
                        elif t == 1 and c >= G:
                            act.wait_ge(s_gps, vm2[(c - G, T - 1)])
                        z2out = M2[i] if t == 0 else z2[i]
                        nc.scalar.activation(out=z2out[:, :], in_=A2p[i][:, :],
                                             func=Act.Copy).then_inc(s_act, 1)
                for c in C:
                    i = c % G
                    act.wait_ge(s_dve, vt2[c])
                    if c >= G:
                        act.wait_ge(s_dma, vdma_om[c - G])
                    nc.scalar.activation(out=outm[i][:, :], in_=t2[i][0:1, :],
                                         func=Act.Tanh)
                    act.wait_ge(s_dma, vdma_ts0[c])
                    if c == 0:
                        act.wait_ge(s_gps, 1)
                    nc.scalar.activation(out=outsa[i][:, :], in_=ts0[i][:, :],
                                         func=Act.Sigmoid, bias=sb_two[0:1, 0:1]) \
                        .then_inc(s_act, 1)

        @block.gpsimd
        def _(gps):
            nc.gpsimd.memset(sb_two.ap(), 2.0).then_inc(s_gps, 1)
            for C in GROUPS:
                for t in range(1, T):
                    for c in C:
                        i = c % G
                        gps.wait_ge(s_act, vz2[(c, t)])
                        gps.wait_ge(s_dve, vW[(c, t)])
                        nc.gpsimd.tensor_tensor(
                            out=M2[i][:, :], in0=z2[i][:, :], in1=W2t[i][:, :],
                            op=Alu.add).then_inc(s_gps, 1)

    return nc


def _prep_legacy(scal, w_fc1, w_rec1, w_fc2, w_rec2, w_mean, w_std):
    import ml_dtypes

    a1, b1, thr1 = scal["a1"], scal["b1"], scal["thr1"]
    a2, b2, thr2 = scal["a2"], scal["b2"], scal["thr2"]
    bf = ml_dtypes.bfloat16
    w1 = (w_rec1.T / thr1).astype(bf)
    r2 = (w_rec2.T / thr2).astype(bf)
    f1s = np.stack([(a1 ** -t) * w_fc1.T for t in range(T)]).astype(bf)
    w2s = np.stack([(a2 ** -t) * (a1 ** (t + 1)) / thr1 * w_fc2.T
                    for t in range(T)]).astype(bf)
    wo = np.concatenate([w_mean, w_std], axis=0).T / (T * thr2)
    wos = np.stack([(a2 ** (t + 1)) * wo for t in range(T)]).astype(bf)
    return w1, r2, f1s, w2s, wos


# ──────────────────────────── entry point ────────────────────────────


def kernel(state, w_fc1, w_rec1, w_fc2, w_rec2, w_mean, w_std,
           alpha1, beta1, thr1, alpha2, beta2, thr2):
    import ml_dtypes
    from concourse.bass_utils import run_bass_kernel_spmd

    state = np.asarray(state, dtype=np.float32)
    scal = {
        "a1": float(np.clip(np.asarray(alpha1, dtype=np.float64), 1e-6, 1.0)),
        "b1": float(np.clip(np.asarray(beta1, dtype=np.float64), 0.0, 1.0)),
        "thr1": float(np.asarray(thr1, dtype=np.float64)),
        "a2": float(np.clip(np.asarray(alpha2, dtype=np.float64), 1e-6, 1.0)),
        "b2": float(np.clip(np.asarray(beta2, dtype=np.float64), 0.0, 1.0)),
        "thr2": float(np.asarray(thr2, dtype=np.float64)),
    }
    fast = (abs(scal["a1"] - scal["b1"]) < 1e-9
            and abs(scal["a2"] - scal["b2"]) < 1e-9)

    ws = (np.asarray(w_fc1, np.float32), np.asarray(w_rec1, np.float32),
          np.asarray(w_fc2, np.float32), np.asarray(w_rec2, np.float32),
          np.asarray(w_mean, np.float32), np.asarray(w_std, np.float32))
    trace = bool(int(os.environ.get("SNN_TRACE", "0")))

    if fast:
        key = ("fast",) + tuple(sorted(scal.items()))
        if key not in _CACHE:
            _CACHE[key] = _build_fast()
        nc = _CACHE[key]
        wbig, fweights, consts = _prep_fast(scal, *ws)
        stateT = state.T.astype(ml_dtypes.bfloat16)  # [6, B_TOTAL]
        in_maps = []
        for c in range(N_CORES):
            fbig = np.empty((SD, FB_W), dtype=ml_dtypes.bfloat16)
            fbig[:, :T * H] = fweights
            fbig[:, T * H:] = stateT[:, c * BC:(c + 1) * BC]
            in_maps.append({"fbig": fbig, "wbig": wbig, "consts": consts})
    else:
        key = ("legacy",) + tuple(sorted(scal.items()))
        if key not in _CACHE:
            _CACHE[key] = _build_legacy(scal)
        nc = _CACHE[key]
        w1, r2, f1s, w2s, wos = _prep_legacy(scal, *ws)
        stateT = state.T.astype(ml_dtypes.bfloat16)
        in_maps = []
        for c in range(N_CORES):
            in_maps.append({
                "stateT": np.ascontiguousarray(stateT[:, c * BC:(c + 1) * BC]),
                "w1": w1, "r2": r2, "f1s": f1s, "w2s": w2s, "wos": wos,
            })

    res = run_bass_kernel_spmd(nc, in_maps, core_ids=list(range(N_CORES)),
                               trace=trace)
    kernel.last_results = res
    vm = np.concatenate([res.results[c]["out_mean"] for c in range(N_CORES)], axis=1)
    vs = np.concatenate([res.results[c]["out_std"] for c in range(N_CORES)], axis=1)
    return vm.reshape(-1, 1), vs.reshape(-1, 1)


# revision 19
# speedup vs baseline: 2.1914x; 1.2681x over previous
"""Trainium2 Bass kernel for a 2-layer spiking (Synaptic) critic network.

Math (per batch row, T=8 steps, H=128, reset-by-subtract from previous spike):
    cur1 = state @ w_fc1.T
    syn1 = a1*syn1 + cur1 + spk1 @ w_rec1.T ; mem1 = b1*mem1 + syn1 - thr1*spk1_prev
    spk1 = (mem1 > thr1) ; layer 2 analogous with inputs spk1 @ w_fc2.T + spk2 @ w_rec2.T
    out_mean = tanh(mean_t(spk2) @ w_mean.T); out_std = 1.9*sigmoid(.. @ w_std.T + 2) + .1

Fast path (requires beta == alpha, which clip-holds for the shipped scalars):

  Work in the a^-t scaled domain M_t = a^-t * mem_t.  With b == a the decay
  multiplier (b/a) is exactly 1, so the scaled membrane is a PURE RUNNING SUM
  and can live in PSUM for all 8 steps, accumulated by the PE:
      M1_t = M1_{t-1} + a1^-t*cur1 + spk1_{t-1} @ [a1^-t (w_rec1.T - thr1 I)]
  The reset-by-subtract is folded into the recurrent weight matrix (the -thr*I
  term), so no elementwise membrane updates exist at all.  Per step each layer
  needs ONLY its matmuls plus one spike-extraction op reading PSUM directly:

    spk1 via ScalarE:  Sh1 = Sign(M1psum + C1adj_t)  in {-1,+1}
      (sign-encoded spikes: spk = (Sh1+1)/2; consumers use half-weights W/2 and
       the constant 1/2-column-sum corrections are folded -- cumulatively, in
       f64 on the host -- into the comparison constants C1adj/C2adj, which are
       per-partition f32 vectors.  Nothing else ever reads M, so the correction
       never needs to be materialized in PSUM.)
    spk2 via VectorE:  S2 = (M2psum + C2adj_t) > 0    in {0,1}

  The output head accumulates  AO = sum_t S2_t @ ([w_mean|w_std].T / 8)  into a
  shared PSUM bank (constant weights; chunk i owns partition pair 32i..32i+1),
  so tanh/sigmoid run once per chunk at the group tail.

  Engine load per chunk-step: PE 5 matmuls (f1, W1h, W2h, R2, wo; all N=512),
  ACT 1 Sign, DVE 1 tensor_scalar.  The PE is the top engine by ~2x, which
  keeps it densely busy and HAM-warm (2.4 GHz) instead of oscillating at the
  cold 1.2 GHz clock like the previous formulation.

  Chunks are processed in interleaved groups of G=3 (each chunk owns M1/M2
  PSUM banks; 3*2+1 = 7 of 8 banks) so every cross-engine wait is covered by
  the other chunks' work.

Raw Bass (no Tile): this walrus build rejects instructions carrying more than
one attached semaphore wait; explicit engine blocks with standalone wait_ge
instructions sidestep the limit.
"""

import os
from contextlib import ExitStack

import numpy as np

N_CORES = 8
B_TOTAL = 65536
BC = B_TOTAL // N_CORES  # 8192 rows per core
CB = 512                 # batch-column chunk (one PSUM bank)
NCHUNK = BC // CB        # 16
G = 3                    # chunks interleaved in flight
T = 8
H = 128
SD = 6

GROUPS = [list(range(g, min(g + G, NCHUNK))) for g in range(0, NCHUNK, G)]

# FBIG column layout: [f1 stack T*H | state BC]
FB_W = T * H + BC
# WBIG column layout: [W1h T*H | W2h T*H | R2 T*H | wo 2]
WB_W = 3 * T * H + 2
# CONSTS column layout: [C2adj T | C1adj T | 2.0]
CN_W = 2 * T + 1

_CACHE: dict = {}


# ───────────────────────────── fast path ─────────────────────────────


def _schedule_fast():
    """Semaphore target values for every event, mirroring emission order.

    PE stream order: per group, steps t=0..7 of [A1 block, A2 block]; the
    group's single tail wo-matmul per chunk (vAOt) is emitted AFTER the NEXT
    group's t=0 blocks, so the PE never idles waiting for the DVE/GPS tail
    chain at a group boundary (the last group's tail follows immediately).
    """
    vA1, vA2, vAOt = {}, {}, {}
    pe = 0
    prev_C = None
    for C in GROUPS:
        for t in range(T):
            for c in C:
                pe += 1
                vA1[(c, t)] = pe
            if t == 0 and prev_C is not None:
                # prev group's tails go between A1 and A2 of t=0: A2 waits on
                # this group's Signs, which ACT only emits after its own group
                # tails -- those need the tail matmuls, so tails must precede A2.
                for c in prev_C:
                    pe += 1
                    vAOt[c] = pe
            for c in C:
                pe += 1
                vA2[(c, t)] = pe
        prev_C = C
    for c in prev_C:
        pe += 1
        vAOt[c] = pe

    vSP = {}
    gp = 0
    for C in GROUPS:
        for t in range(T):
            for c in C:
                gp += 1
                vSP[(c, t)] = gp

    vS1h, vtail = {}, {}
    ac = 0
    for C in GROUPS:
        for t in range(T):
            for c in C:
                ac += 1
                vS1h[(c, t)] = ac
        for c in C:
            ac += 1
            vtail[c] = ac

    vS2, vt2, vouts = {}, {}, {}
    dv = 0
    for C in GROUPS:
        for t in range(T):
            for c in C:
                dv += 1
                vS2[(c, t)] = dv
        for c in C:
            dv += 1
            vt2[c] = dv
        for c in C:
            dv += 1
            vouts[c] = dv

    N_INIT = 3  # fbig, wbig, consts
    vdma_ts0, vdma_om, vdma_os = {}, {}, {}
    dm = N_INIT
    for C in GROUPS:
        for c in C:
            dm += 1
            vdma_ts0[c] = dm * 16
        for c in C:
            dm += 1
            vdma_om[c] = dm * 16
            dm += 1
            vdma_os[c] = dm * 16
    return dict(vA1=vA1, vA2=vA2, vAOt=vAOt, vSP=vSP, vS1h=vS1h, vtail=vtail,
                vS2=vS2, vt2=vt2, vouts=vouts, vdma_ts0=vdma_ts0,
                vdma_om=vdma_om, vdma_os=vdma_os)


def _build_fast():
    import concourse.bass as bass
    import concourse.mybir as mybir

    f32 = mybir.dt.float32
    bf16 = mybir.dt.bfloat16
    Alu = mybir.AluOpType
    Act = mybir.ActivationFunctionType

    S = _schedule_fast()
    vA1, vA2, vAOt, vSP = S["vA1"], S["vA2"], S["vAOt"], S["vSP"]
    vS1h, vtail = S["vS1h"], S["vtail"]
    vS2, vt2, vouts = S["vS2"], S["vt2"], S["vouts"]
    vdma_ts0, vdma_om, vdma_os = S["vdma_ts0"], S["vdma_om"], S["vdma_os"]

    nc = bass.Bass()
    d_fbig = nc.declare_dram_parameter("fbig", [SD, FB_W], bf16, isOutput=False)
    d_wbig = nc.declare_dram_parameter("wbig", [H, WB_W], bf16, isOutput=False)
    d_cn = nc.declare_dram_parameter("consts", [H, CN_W], f32, isOutput=False)
    d_om = nc.declare_dram_parameter("out_mean", [1, BC], f32, isOutput=True)
    d_os = nc.declare_dram_parameter("out_std", [1, BC], f32, isOutput=True)

    with ExitStack() as ctx:
        E = ctx.enter_context
        fb = E(nc.sbuf_tensor([SD, FB_W], bf16))
        wb = E(nc.sbuf_tensor([H, WB_W], bf16))
        cn = E(nc.sbuf_tensor([H, CN_W], f32))

        Sh1 = [E(nc.sbuf_tensor(f"Sh1_{i}", [H, CB], bf16)) for i in range(G)]
        S2 = [E(nc.sbuf_tensor(f"S2_{i}", [H, CB], bf16)) for i in range(G)]
        SP = [E(nc.sbuf_tensor(f"SP_{i}", [H, CB], bf16)) for i in range(G)]
        t2 = [E(nc.sbuf_tensor(f"t2_{i}", [2, CB], f32)) for i in range(G)]
        ts0 = [E(nc.sbuf_tensor(f"ts0_{i}", [1, CB], f32)) for i in range(G)]
        outm = [E(nc.sbuf_tensor(f"outm_{i}", [1, CB], f32)) for i in range(G)]
        sa = [E(nc.sbuf_tensor(f"sa_{i}", [1, CB], f32)) for i in range(G)]
        outs = [E(nc.sbuf_tensor(f"outs_{i}", [1, CB], f32)) for i in range(G)]

        M1p = [E(nc.psum_tensor(f"M1_{i}", [H, CB], f32)) for i in range(G)]
        M2p = [E(nc.psum_tensor(f"M2_{i}", [H, CB], f32)) for i in range(G)]
        AOp = E(nc.psum_tensor("AO", [H, CB], f32))  # chunk i: rows 32i..32i+1

        s_pe = E(nc.semaphore("s_pe"))
        s_dve = E(nc.semaphore("s_dve"))
        s_act = E(nc.semaphore("s_act"))
        s_gps = E(nc.semaphore("s_gps"))
        s_dma = E(nc.semaphore("s_dma"))

        def w1h(t):
            return wb[:, t * H:(t + 1) * H]

        def w2h(t):
            return wb[:, T * H + t * H:T * H + (t + 1) * H]

        def r2w(t):
            return wb[:, 2 * T * H + t * H:2 * T * H + (t + 1) * H]

        wo = wb[:, 3 * T * H:3 * T * H + 2]

        def f1w(t):
            return fb[:, t * H:(t + 1) * H]

        def st(c):
            return fb[:, T * H + c * CB:T * H + (c + 1) * CB]

        block = E(nc.Block())

        @block.sync
        def _(sp):
            sp.dma_start(out=fb[:, :], in_=d_fbig[:, :]).then_inc(s_dma, 16)
            sp.dma_start(out=wb[:, :], in_=d_wbig[:, :]).then_inc(s_dma, 16)
            sp.dma_start(out=cn[:, :], in_=d_cn[:, :]).then_inc(s_dma, 16)
            for C in GROUPS:
                for c in C:
                    i = c % G
                    sp.wait_ge(s_dve, vt2[c])
                    sp.dma_start(out=ts0[i][:, :], in_=t2[i][1:2, :]) \
                        .then_inc(s_dma, 16)
                for c in C:
                    i = c % G
                    cs = slice(c * CB, (c + 1) * CB)
                    sp.wait_ge(s_act, vtail[c])
                    sp.dma_start(out=d_om[0:1, cs], in_=outm[i][:, :]) \
                        .then_inc(s_dma, 16)
                    sp.wait_ge(s_dve, vouts[c])
                    sp.dma_start(out=d_os[0:1, cs], in_=outs[i][:, :]) \
                        .then_inc(s_dma, 16)

        @block.tensor
        def _(pe):
            def tail_mm(c):
                i = c % G
                pe.wait_ge(s_gps, vSP[(c, T - 1)])
                if c >= G:
                    pe.wait_ge(s_dve, vt2[c - G])  # AOp pair release
                nc.tensor.matmul(AOp[32 * i:32 * i + 2, :], wo, SP[i][:, :],
                                 start=True, stop=True,
                                 skip_group_check=True).then_inc(s_pe, 1)

            prev_C = None
            for C in GROUPS:
                for t in range(T):
                    last = t == T - 1
                    for c in C:
                        i = c % G
                        if t >= 1:
                            pe.wait_ge(s_act, vS1h[(c, t - 1)])
                        elif c >= G:
                            pe.wait_ge(s_act, vS1h[(c - G, T - 1)])
                        elif c == 0:
                            pe.wait_ge(s_dma, 16)
                        if t >= 1:
                            nc.tensor.matmul(M1p[i][:, :], w1h(t), Sh1[i][:, :],
                                             start=False, stop=False,
                                             skip_group_check=True)
                        nc.tensor.matmul(M1p[i][:, :], f1w(t), st(c),
                                         start=(t == 0), stop=last,
                                         skip_group_check=True) \
                            .then_inc(s_pe, 1)
                    if t == 0 and prev_C is not None:
                        for c in prev_C:
                            tail_mm(c)
                    for c in C:
                        i = c % G
                        pe.wait_ge(s_act, vS1h[(c, t)])
                        if t >= 1:
                            pe.wait_ge(s_dve, vS2[(c, t - 1)])
                        elif c >= G:
                            pe.wait_ge(s_dve, vS2[(c - G, T - 1)])
                        elif c == 0:
                            pe.wait_ge(s_dma, 32)
                        if t >= 1:
                            nc.tensor.matmul(M2p[i][:, :], w2h(t), Sh1[i][:, :],
                                             start=False, stop=False,
                                             skip_group_check=True)
                            nc.tensor.matmul(M2p[i][:, :], r2w(t), S2[i][:, :],
                                             start=False, stop=last,
                                             skip_group_check=True) \
                                .then_inc(s_pe, 1)
                        else:
                            nc.tensor.matmul(M2p[i][:, :], w2h(t), Sh1[i][:, :],
                                             start=True, stop=False,
                                             skip_group_check=True) \
                                .then_inc(s_pe, 1)
                prev_C = C
            for c in prev_C:
                tail_mm(c)

        @block.gpsimd
        def _(gps):
            for C in GROUPS:
                for t in range(T):
                    for c in C:
                        i = c % G
                        gps.wait_ge(s_dve, vS2[(c, t)])
                        if t == 0:
                            if c >= G:
                                gps.wait_ge(s_pe, vAOt[c - G])  # SP release
                            nc.gpsimd.tensor_copy(out=SP[i][:, :],
                                                  in_=S2[i][:, :]) \
                                .then_inc(s_gps, 1)
                        else:
                            nc.gpsimd.tensor_tensor(
                                out=SP[i][:, :], in0=SP[i][:, :],
                                in1=S2[i][:, :], op=Alu.add).then_inc(s_gps, 1)

        @block.scalar
        def _(act):
            act.wait_ge(s_dma, 48)
            for C in GROUPS:
                for t in range(T):
                    for c in C:
                        i = c % G
                        act.wait_ge(s_pe, vA1[(c, t)])
                        nc.scalar.activation(
                            out=Sh1[i][:, :], in_=M1p[i][:, :], func=Act.Sign,
                            bias=cn[:, T + t:T + t + 1]).then_inc(s_act, 1)
                for c in C:
                    i = c % G
                    act.wait_ge(s_dve, vt2[c])
                    if c >= G:
                        act.wait_ge(s_dma, vdma_om[c - G])
                    nc.scalar.activation(out=outm[i][:, :], in_=t2[i][0:1, :],
                                         func=Act.Tanh)
                    act.wait_ge(s_dma, vdma_ts0[c])
                    if c >= G:
                        act.wait_ge(s_dve, vouts[c - G])
                    nc.scalar.activation(out=sa[i][:, :], in_=ts0[i][:, :],
                                         func=Act.Sigmoid,
                                         bias=cn[0:1, 2 * T:2 * T + 1]) \
                        .then_inc(s_act, 1)

        @block.vector
        def _(dve):
            dve.wait_ge(s_dma, 48)
            for C in GROUPS:
                for t in range(T):
                    for c in C:
                        i = c % G
                        dve.wait_ge(s_pe, vA2[(c, t)])
                        if t >= 1:
                            dve.wait_ge(s_gps, vSP[(c, t - 1)])
                        elif c >= G:
                            dve.wait_ge(s_gps, vSP[(c - G, T - 1)])
                        nc.vector.tensor_scalar(
                            out=S2[i][:, :], in0=M2p[i][:, :],
                            scalar1=cn[:, t:t + 1], scalar2=0.0,
                            op0=Alu.add, op1=Alu.is_gt).then_inc(s_dve, 1)
                for c in C:
                    i = c % G
                    dve.wait_ge(s_pe, vAOt[c])
                    if c >= G:
                        dve.wait_ge(s_dma, vdma_ts0[c - G])
                        dve.wait_ge(s_act, vtail[c - G])
                    nc.vector.tensor_copy(out=t2[i][:, :],
                                          in_=AOp[32 * i:32 * i + 2, :]) \
                        .then_inc(s_dve, 1)
                for c in C:
                    i = c % G
                    dve.wait_ge(s_act, vtail[c])
                    if c >= G:
                        dve.wait_ge(s_dma, vdma_os[c - G])
                    nc.vector.tensor_scalar(
                        out=outs[i][:, :], in0=sa[i][:, :],
                        scalar1=1.9, scalar2=0.1, op0=Alu.mult, op1=Alu.add,
                    ).then_inc(s_dve, 1)

    return nc


def _prep_fast(scal, w_fc1, w_rec1, w_fc2, w_rec2, w_mean, w_std):
    """Host-side weight/constant packing (f64 precision)."""
    import ml_dtypes

    bf = ml_dtypes.bfloat16
    a1, thr1 = scal["a1"], scal["thr1"]
    a2, thr2 = scal["a2"], scal["thr2"]
    Iden = np.eye(H, dtype=np.float64)

    w1T = w_rec1.T.astype(np.float64)
    w2T = w_fc2.T.astype(np.float64)
    r2T = w_rec2.T.astype(np.float64)
    f1T = w_fc1.T.astype(np.float64)

    wbig = np.zeros((H, WB_W), dtype=np.float64)
    consts = np.zeros((H, CN_W), dtype=np.float64)
    fweights = np.zeros((SD, T * H), dtype=np.float64)

    C1 = np.zeros(H, dtype=np.float64)
    C2 = np.zeros(H, dtype=np.float64)
    for t in range(T):
        s1 = a1 ** (-t)
        s2 = a2 ** (-t)
        fweights[:, t * H:(t + 1) * H] = s1 * f1T
        if t >= 1:
            W1 = s1 * (w1T - thr1 * Iden)
            wbig[:, t * H:(t + 1) * H] = 0.5 * W1
            C1 = C1 + 0.5 * W1.sum(axis=0)
            wbig[:, 2 * T * H + t * H:2 * T * H + (t + 1) * H] = \
                s2 * (r2T - thr2 * Iden)
        W2 = s2 * w2T
        wbig[:, T * H + t * H:T * H + (t + 1) * H] = 0.5 * W2
        C2 = C2 + 0.5 * W2.sum(axis=0)
        consts[:, t] = C2 - thr2 * s2
        consts[:, T + t] = C1 - thr1 * s1
    wbig[:, 3 * T * H:3 * T * H + 2] = \
        np.concatenate([w_mean, w_std], axis=0).T / T
    consts[:, 2 * T] = 2.0

    return (wbig.astype(bf), fweights.astype(bf), consts.astype(np.float32))


# ──────────────────────────── legacy path ────────────────────────────
# General (beta != alpha) fallback: the previous pipelined formulation.


def _schedule_legacy():
    vA1, vA2, vAO = {}, {}, {}
    pe = 0
    for C in GROUPS:
        for t in range(T):
            for c in C:
                pe += 1
                vA1[(c, t)] = pe
            for c in C:
                pe += 1
                vA2[(c, t)] = pe
            for c in C:
                pe += 1
                vAO[(c, t)] = pe

    vW, vS1, vS2, vt2, vouts = {}, {}, {}, {}, {}
    dv = 0
    for C in GROUPS:
        for t in range(T):
            for c in C:
                dv += 1
                vW[(c, t)] = dv
            for c in C:
                dv += 1
                vS1[(c, t)] = dv
            for c in C:
                dv += 1
                vS2[(c, t)] = dv
        for c in C:
            dv += 1
            vt2[c] = dv
        for c in C:
            dv += 1
            vouts[c] = dv

    vz1, vz2, vsig = {}, {}, {}
    ac = 0
    for C in GROUPS:
        for t in range(T):
            for c in C:
                ac += 1
                vz1[(c, t)] = ac
            for c in C:
                ac += 1
                vz2[(c, t)] = ac
        for c in C:
            ac += 1
            vsig[c] = ac

    vm2 = {}
    gp = 1
    for C in GROUPS:
        for t in range(1, T):
            for c in C:
                gp += 1
                vm2[(c, t)] = gp

    N_INIT = 3 + 3 * T
    vdma_ts0, vdma_om, vdma_os = {}, {}, {}
    dm = N_INIT
    for C in GROUPS:
        for c in C:
            dm += 1
            vdma_ts0[c] = dm * 16
        for c in C:
            dm += 1
            vdma_om[c] = dm * 16
            dm += 1
            vdma_os[c] = dm * 16
    return dict(vA1=vA1, vA2=vA2, vAO=vAO, vW=vW, vS1=vS1, vS2=vS2, vt2=vt2,
                vouts=vouts, vz1=vz1, vz2=vz2, vsig=vsig, vm2=vm2,
                N_INIT=N_INIT, vdma_ts0=vdma_ts0, vdma_om=vdma_om,
                vdma_os=vdma_os)


def _build_legacy(scal):
    import concourse.bass as bass
    import concourse.mybir as mybir

    a1, b1, thr1 = scal["a1"], scal["b1"], scal["thr1"]
    a2, b2, thr2 = scal["a2"], scal["b2"], scal["thr2"]
    f32 = mybir.dt.float32
    bf16 = mybir.dt.bfloat16
    Alu = mybir.AluOpType
    Act = mybir.ActivationFunctionType

    S = _schedule_legacy()
    vA1, vA2, vAO = S["vA1"], S["vA2"], S["vAO"]
    vW, vS1, vS2 = S["vW"], S["vS1"], S["vS2"]
    vt2, vouts = S["vt2"], S["vouts"]
    vz1, vz2, vsig = S["vz1"], S["vz2"], S["vsig"]
    vm2 = S["vm2"]
    N_INIT = S["N_INIT"]
    vdma_ts0, vdma_om, vdma_os = S["vdma_ts0"], S["vdma_om"], S["vdma_os"]

    nc = bass.Bass()
    d_state = nc.declare_dram_parameter("stateT", [SD, BC], bf16, isOutput=False)
    d_w1 = nc.declare_dram_parameter("w1", [H, H], bf16, isOutput=False)
    d_r2 = nc.declare_dram_parameter("r2", [H, H], bf16, isOutput=False)
    d_f1 = nc.declare_dram_parameter("f1s", [T, SD, H], bf16, isOutput=False)
    d_w2 = nc.declare_dram_parameter("w2s", [T, H, H], bf16, isOutput=False)
    d_wo = nc.declare_dram_parameter("wos", [T, H, 2], bf16, isOutput=False)
    d_om = nc.declare_dram_parameter("out_mean", [1, BC], f32, isOutput=True)
    d_os = nc.declare_dram_parameter("out_std", [1, BC], f32, isOutput=True)

    with ExitStack() as ctx:
        E = ctx.enter_context
        sb_state = E(nc.sbuf_tensor([SD, BC], bf16))
        sb_w1 = E(nc.sbuf_tensor([H, H], bf16))
        sb_r2 = E(nc.sbuf_tensor([H, H], bf16))
        sb_f1 = E(nc.sbuf_tensor([SD, T, H], bf16))
        sb_w2 = E(nc.sbuf_tensor([H, T, H], bf16))
        sb_wo = E(nc.sbuf_tensor([H, T, 2], bf16))
        sb_two = E(nc.sbuf_tensor([1, 1], f32))

        M1 = [E(nc.sbuf_tensor(f"M1_{i}", [H, CB], bf16)) for i in range(G)]
        S1 = [E(nc.sbuf_tensor(f"S1_{i}", [H, CB], bf16)) for i in range(G)]
        W1t = [E(nc.sbuf_tensor(f"W1t_{i}", [H, CB], bf16)) for i in range(G)]
        z1 = [E(nc.sbuf_tensor(f"z1_{i}", [H, CB], bf16)) for i in range(G)]
        M2 = [E(nc.sbuf_tensor(f"M2_{i}", [H, CB], bf16)) for i in range(G)]
        S2 = [E(nc.sbuf_tensor(f"S2_{i}", [H, CB], bf16)) for i in range(G)]
        W2t = [E(nc.sbuf_tensor(f"W2t_{i}", [H, CB], bf16)) for i in range(G)]
        z2 = [E(nc.sbuf_tensor(f"z2_{i}", [H, CB], bf16)) for i in range(G)]
        t2 = [E(nc.sbuf_tensor(f"t2_{i}", [2, CB], f32)) for i in range(G)]
        ts0 = [E(nc.sbuf_tensor(f"ts0_{i}", [1, CB], f32)) for i in range(G)]
        outm = [E(nc.sbuf_tensor(f"outm_{i}", [1, CB], f32)) for i in range(G)]
        outsa = [E(nc.sbuf_tensor(f"outsa_{i}", [1, CB], f32)) for i in range(G)]
        outs2 = [E(nc.sbuf_tensor(f"outs2_{i}", [1, CB], f32)) for i in range(G)]

        A1p = [E(nc.psum_tensor(f"A1_{i}", [H, CB], f32)) for i in range(G)]
        A2p = [E(nc.psum_tensor(f"A2_{i}", [H, CB], f32)) for i in range(G)]
        AOp = E(nc.psum_tensor("AO", [H, CB], f32))

        s_pe = E(nc.semaphore("s_pe"))
        s_dve = E(nc.semaphore("s_dve"))
        s_act = E(nc.semaphore("s_act"))
        s_gps = E(nc.semaphore("s_gps"))
        s_dma = E(nc.semaphore("s_dma"))

        block = E(nc.Block())

        @block.sync
        def _(sp):
            sp.dma_start(out=sb_state[:, :], in_=d_state[:, :]).then_inc(s_dma, 16)
            sp.dma_start(out=sb_w1[:, :], in_=d_w1[:, :]).then_inc(s_dma, 16)
            sp.dma_start(out=sb_r2[:, :], in_=d_r2[:, :]).then_inc(s_dma, 16)
            for t in range(T):
                sp.dma_start(out=sb_f1[:, t, :], in_=d_f1[t, :, :]).then_inc(s_dma, 16)
                sp.dma_start(out=sb_w2[:, t, :], in_=d_w2[t, :, :]).then_inc(s_dma, 16)
                sp.dma_start(out=sb_wo[:, t, :], in_=d_wo[t, :, :]).then_inc(s_dma, 16)
            for C in GROUPS:
                for c in C:
                    i = c % G
                    sp.wait_ge(s_dve, vt2[c])
                    sp.dma_start(out=ts0[i][:, :], in_=t2[i][1:2, :]) \
                        .then_inc(s_dma, 16)
                for c in C:
                    i = c % G
                    cs = slice(c * CB, (c + 1) * CB)
                    sp.wait_ge(s_act, vsig[c])
                    sp.dma_start(out=d_om[0:1, cs], in_=outm[i][:, :]) \
                        .then_inc(s_dma, 16)
                    sp.wait_ge(s_dve, vouts[c])
                    sp.dma_start(out=d_os[0:1, cs], in_=outs2[i][:, :]) \
                        .then_inc(s_dma, 16)

        @block.tensor
        def _(pe):
            pe.wait_ge(s_dma, N_INIT * 16)
            for C in GROUPS:
                for t in range(T):
                    last = t == T - 1
                    for c in C:
                        i = c % G
                        if t >= 1:
                            pe.wait_ge(s_dve, vS1[(c, t - 1)])
                            pe.wait_ge(s_act, vz1[(c, t - 1)])
                        elif c >= G:
                            pe.wait_ge(s_act, vz1[(c - G, T - 1)])
                        if t >= 1:
                            nc.tensor.matmul(A1p[i][:, :], sb_w1[:, :], S1[i][:, :],
                                             start=False, stop=False,
                                             skip_group_check=True)
                        nc.tensor.matmul(A1p[i][:, :], sb_f1[:, t, :],
                                         sb_state[:, c * CB:(c + 1) * CB],
                                         start=(t == 0), stop=last,
                                         skip_group_check=True) \
                            .then_inc(s_pe, 1)
                    for c in C:
                        i = c % G
                        pe.wait_ge(s_dve, vS1[(c, t)])
                        if t >= 1:
                            pe.wait_ge(s_act, vz2[(c, t - 1)])
                        elif c >= G:
                            pe.wait_ge(s_act, vz2[(c - G, T - 1)])
                        if t >= 1:
                            nc.tensor.matmul(A2p[i][:, :], sb_r2[:, :], S2[i][:, :],
                                             start=False, stop=False,
                                             skip_group_check=True)
                        nc.tensor.matmul(A2p[i][:, :], sb_w2[:, t, :], S1[i][:, :],
                                         start=(t == 0), stop=last,
                                         skip_group_check=True) \
                            .then_inc(s_pe, 1)
                    for c in C:
                        i = c % G
                        pe.wait_ge(s_dve, vS2[(c, t)])
                        nc.tensor.matmul(AOp[32 * i:32 * i + 2, :],
                                         sb_wo[:, t, :], S2[i][:, :],
                                         start=(t == 0), stop=last,
                                         skip_group_check=True) \
                            .then_inc(s_pe, 1)

        @block.vector
        def _(dve):
            for C in GROUPS:
                for t in range(T):
                    for c in C:
                        i = c % G
                        if t >= 1:
                            nc.vector.scalar_tensor_tensor(
                                out=W1t[i][:, :], in0=M1[i][:, :], scalar=b1 / a1,
                                in1=S1[i][:, :], op0=Alu.mult, op1=Alu.subtract)
                            if t >= 2:
                                dve.wait_ge(s_gps, vm2[(c, t - 1)])
                            elif c >= G:
                                dve.wait_ge(s_gps, vm2[(c - G, T - 1)])
                            nc.vector.scalar_tensor_tensor(
                                out=W2t[i][:, :], in0=M2[i][:, :], scalar=b2 / a2,
                                in1=S2[i][:, :], op0=Alu.mult, op1=Alu.subtract,
                            ).then_inc(s_dve, 1)
                        else:
                            dve.wait_ge(s_act, vz1[(c, t)])
                            nc.vector.tensor_copy(
                                out=W1t[i][0:1, 0:1], in_=M1[i][0:1, 0:1]
                            ).then_inc(s_dve, 1)
                    for c in C:
                        i = c % G
                        at1 = a1 ** (-t)
                        if t >= 1:
                            dve.wait_ge(s_act, vz1[(c, t)])
                            nc.vector.tensor_tensor(
                                out=M1[i][:, :], in0=z1[i][:, :], in1=W1t[i][:, :],
                                op=Alu.add)
                        nc.vector.tensor_scalar(
                            out=S1[i][:, :], in0=M1[i][:, :],
                            scalar1=thr1 * at1, scalar2=thr1 * at1 / a1,
                            op0=Alu.is_gt, op1=Alu.mult,
                        ).then_inc(s_dve, 1)
                    for c in C:
                        i = c % G
                        at2 = a2 ** (-t)
                        if t >= 1:
                            dve.wait_ge(s_gps, vm2[(c, t)])
                        else:
                            dve.wait_ge(s_act, vz2[(c, t)])
                        nc.vector.tensor_scalar(
                            out=S2[i][:, :], in0=M2[i][:, :],
                            scalar1=thr2 * at2, scalar2=thr2 * at2 / a2,
                            op0=Alu.is_gt, op1=Alu.mult,
                        ).then_inc(s_dve, 1)
                for c in C:
                    i = c % G
                    dve.wait_ge(s_pe, vAO[(C[-1], T - 1)])
                    if c >= G:
                        dve.wait_ge(s_dma, vdma_ts0[c - G])
                    nc.vector.tensor_copy(out=t2[i][:, :],
                                          in_=AOp[32 * i:32 * i + 2, :]) \
                        .then_inc(s_dve, 1)
                for c in C:
                    i = c % G
                    dve.wait_ge(s_act, vsig[c])
                    if c >= G:
                        dve.wait_ge(s_dma, vdma_os[c - G])
                    nc.vector.tensor_scalar(
                        out=outs2[i][:, :], in0=outsa[i][:, :],
                        scalar1=1.9, scalar2=0.1, op0=Alu.mult, op1=Alu.add,
                    ).then_inc(s_dve, 1)

        @block.scalar
        def _(act):
            for C in GROUPS:
                for t in range(T):
                    for c in C:
                        i = c % G
                        act.wait_ge(s_pe, vA1[(c, t)])
                        if t >= 1:
                            act.wait_ge(s_dve, vS1[(c, t - 1)])
                        elif c >= G:
                            act.wait_ge(s_dve, vt2[c - G])
                        z1out = M1[i] if t == 0 else z1[i]
                        nc.scalar.activation(out=z1out[:, :], in_=A1p[i][:, :],
                                             func=Act.Copy).then_inc(s_act, 1)
                    for c in C:
                        i = c % G
                        act.wait_ge(s_pe, vA2[(c, t)])
                        if t >= 2:
                            act.wait_ge(s_gps, vm2[(c, t - 1)])
                        elif t == 1 and c >= G:
                            act.wait_ge(s_gps, vm2[(c - G, T - 1)])
                        z2out = M2[i] if t == 0 else z2[i]
                        nc.scalar.activation(out=z2out[:, :], in_=A2p[i][:, :],
                                             func=Act.Copy).then_inc(s_act, 1)
                for c in C:
                    i = c % G
                    act.wait_ge(s_dve, vt2[c])
                    if c >= G:
                        act.wait_ge(s_dma, vdma_om[c - G])
                    nc.scalar.activation(out=outm[i][:, :], in_=t2[i][0:1, :],
                                         func=Act.Tanh)
                    act.wait_ge(s_dma, vdma_ts0[c])
                    if c == 0:
                        act.wait_ge(s_gps, 1)
                    nc.scalar.activation(out=outsa[i][:, :], in_=ts0[i][:, :],
                                         func=Act.Sigmoid, bias=sb_two[0:1, 0:1]) \
                        .then_inc(s_act, 1)

        @block.gpsimd
        def _(gps):
            nc.gpsimd.memset(sb_two.ap(), 2.0).then_inc(s_gps, 1)
            for C in GROUPS:
                for t in range(1, T):
                    for c in C:
                        i = c % G
                        gps.wait_ge(s_act, vz2[(c, t)])
                        gps.wait_ge(s_dve, vW[(c, t)])
                        nc.gpsimd.tensor_tensor(
                            out=M2[i][:, :], in0=z2[i][:, :], in1=W2t[i][:, :],
                            op=Alu.add).then_inc(s_gps, 1)

    return nc


def _prep_legacy(scal, w_fc1, w_rec1, w_fc2, w_rec2, w_mean, w_std):
    import ml_dtypes

    a1, b1, thr1 = scal["a1"], scal["b1"], scal["thr1"]
    a2, b2, thr2 = scal["a2"], scal["b2"], scal["thr2"]
    bf = ml_dtypes.bfloat16
    w1 = (w_rec1.T / thr1).astype(bf)
    r2 = (w_rec2.T / thr2).astype(bf)
    f1s = np.stack([(a1 ** -t) * w_fc1.T for t in range(T)]).astype(bf)
    w2s = np.stack([(a2 ** -t) * (a1 ** (t + 1)) / thr1 * w_fc2.T
                    for t in range(T)]).astype(bf)
    wo = np.concatenate([w_mean, w_std], axis=0).T / (T * thr2)
    wos = np.stack([(a2 ** (t + 1)) * wo for t in range(T)]).astype(bf)
    return w1, r2, f1s, w2s, wos


# ──────────────────────────── entry point ────────────────────────────


def kernel(state, w_fc1, w_rec1, w_fc2, w_rec2, w_mean, w_std,
           alpha1, beta1, thr1, alpha2, beta2, thr2):
    import ml_dtypes
    from concourse.bass_utils import run_bass_kernel_spmd

    state = np.asarray(state, dtype=np.float32)
    scal = {
        "a1": float(np.clip(np.asarray(alpha1, dtype=np.float64), 1e-6, 1.0)),
        "b1": float(np.clip(np.asarray(beta1, dtype=np.float64), 0.0, 1.0)),
        "thr1": float(np.asarray(thr1, dtype=np.float64)),
        "a2": float(np.clip(np.asarray(alpha2, dtype=np.float64), 1e-6, 1.0)),
        "b2": float(np.clip(np.asarray(beta2, dtype=np.float64), 0.0, 1.0)),
        "thr2": float(np.asarray(thr2, dtype=np.float64)),
    }
    fast = (abs(scal["a1"] - scal["b1"]) < 1e-9
            and abs(scal["a2"] - scal["b2"]) < 1e-9)

    ws = (np.asarray(w_fc1, np.float32), np.asarray(w_rec1, np.float32),
          np.asarray(w_fc2, np.float32), np.asarray(w_rec2, np.float32),
          np.asarray(w_mean, np.float32), np.asarray(w_std, np.float32))
    trace = bool(int(os.environ.get("SNN_TRACE", "0")))

    if fast:
        key = ("fast",) + tuple(sorted(scal.items()))
        if key not in _CACHE:
            _CACHE[key] = _build_fast()
        nc = _CACHE[key]
        wbig, fweights, consts = _prep_fast(scal, *ws)
        stateT = state.T.astype(ml_dtypes.bfloat16)  # [6, B_TOTAL]
        in_maps = []
        for c in range(N_CORES):
            fbig = np.empty((SD, FB_W), dtype=ml_dtypes.bfloat16)
            fbig[:, :T * H] = fweights
            fbig[:, T * H:] = stateT[:, c * BC:(c + 1) * BC]
            in_maps.append({"fbig": fbig, "wbig": wbig, "consts": consts})
    else:
        key = ("legacy",) + tuple(sorted(scal.items()))
        if key not in _CACHE:
            _CACHE[key] = _build_legacy(scal)
        nc = _CACHE[key]
        w1, r2, f1s, w2s, wos = _prep_legacy(scal, *ws)
        stateT = state.T.astype(ml_dtypes.bfloat16)
        in_maps = []
        for c in range(N_CORES):
            in_maps.append({
                "stateT": np.ascontiguousarray(stateT[:, c * BC:(c + 1) * BC]),
                "w1": w1, "r2": r2, "f1s": f1s, "w2s": w2s, "wos": wos,
            })

    res = run_bass_kernel_spmd(nc, in_maps, core_ids=list(range(N_CORES)),
                               trace=trace)
    kernel.last_results = res
    vm = np.concatenate([res.results[c]["out_mean"] for c in range(N_CORES)], axis=1)
    vs = np.concatenate([res.results[c]["out_std"] for c in range(N_CORES)], axis=1)
    return vm.reshape(-1, 1), vs.reshape(-1, 1)
